# revision 3
# baseline (speedup 1.0000x reference)
"""Trainium2 Bass kernel for grouped VQ codebook quantize + uniform loss.

Reference semantics (see problem):
  - x [8192, 256] f32, Q [8192] int, E [3328, 256] f32 (26 types x 128 codes), tau scalar
  - quantized[n] = softmax(x[n] @ E_type(Q[n]).T / tau) @ E_type(Q[n])
  - encoding_indices[n] = argmax(softmax) + Q[n]*128
  - uniform_loss = -mean over rows i of log(pos_i / tot_i) where
      sim = En @ En.T (En row-normalized E), diag masked, exp(sim/0.07),
      pos_i = sum over same-type j != i, tot_i = sum over all j != i
  - loss = 0.0

Strategy (8 NeuronCores, SPMD, one NEFF):
  - Quantize: shard the 26 PTM types across cores (4 type-slots/core, host
    groups tokens by type, pads each type to S token slots). Codes live on
    the partition axis: dotT = E_t.T^T @ xT (fp32 matmul, argmax-exact),
    exp (no max-subtraction needed: logits ~ N(0,1)), z_q via one matmul
    against [E_t | 1] (ones column yields the softmax denominator), argmax
    via PE-transpose + DVE max_index.
  - Uniform loss: shard the 26 row-blocks of En @ En.T across cores
    (4 block-slots/core). Each core gets a column-permuted EnT (bf16) with
    its own 4 blocks first, so window/diag handling is core-uniform:
    diagonal killed with copy_predicated before exp, row sums accumulated
    for free via activation accum_out. Host does the final tiny log/mean
    all-reduce.
"""

import numpy as np
import ml_dtypes

NUM_TYPES = 26
KCODES = 128
D = 256
NTOK = 8192
TEMPERATURE = 0.07
NCORES = 8
TSLOTS = 4  # type-slots (quantize) and block-slots (uniform) per core
NE = NUM_TYPES * KCODES  # 3328
UCHUNK = 512
NCHUNKS = (NE + UCHUNK - 1) // UCHUNK  # 7 (6x512 + 1x256)

# slot -> global type/block id per core; -1 = dummy (compute discarded)
SLOT_MAP = [[c + 8 * s if c + 8 * s < NUM_TYPES else -1 for s in range(TSLOTS)]
            for c in range(NCORES)]

_cache = {}


def _build(S):
    import concourse.bacc as bacc
    import concourse.mybir as mybir
    from concourse.tile import TileContext
    from concourse import masks

    F32 = mybir.dt.float32
    BF16 = mybir.dt.bfloat16
    U32 = mybir.dt.uint32
    EXP = mybir.ActivationFunctionType.Exp
    TS = TSLOTS * S
    NCH = S // 128

    nc = bacc.Bacc("TRN2", name="vqcb", num_devices=NCORES, debug=False)
    xT_d = nc.dram_tensor("xT", [2, 128, TS], F32, kind="ExternalInput")
    esel_d = nc.dram_tensor("esel", [TSLOTS, 128, D + 1], F32, kind="ExternalInput")
    etsel_d = nc.dram_tensor("etsel", [2, 128, TSLOTS * KCODES], F32, kind="ExternalInput")
    entp_d = nc.dram_tensor("entp", [2, 128, NE], BF16, kind="ExternalInput")
    zq_d = nc.dram_tensor("zq", [TSLOTS, S, D], F32, kind="ExternalOutput")
    idx_d = nc.dram_tensor("idx", [TSLOTS, NCH, 128], U32, kind="ExternalOutput")
    uout_d = nc.dram_tensor("uout", [128, 2 * TSLOTS], F32, kind="ExternalOutput")

    with TileContext(nc) as tc:
        with (
            tc.tile_pool(name="cpool", bufs=1) as cpool,
            tc.tile_pool(name="wq", bufs=3) as wq,
            tc.tile_pool(name="uw", bufs=3) as uw,
            tc.tile_pool(name="psq", bufs=2, space="PSUM") as psq,
            tc.tile_pool(name="psu", bufs=2, space="PSUM") as psu,
        ):
            # ---- constants / inputs resident in SBUF ----
            idf32 = cpool.tile([128, 128], F32)
            masks.make_identity(nc, idf32[:, :])
            eye8 = cpool.tile([128, 128], mybir.dt.int8)
            masks.make_identity(nc, eye8[:, :])
            negt = cpool.tile([128, 128], F32)
            nc.vector.memset(negt[:, :], -100.0)

            xts = []
            etl = []
            for j in range(2):
                xt = cpool.tile([128, TS], F32, name=f"xt{j}", tag=f"xt{j}")
                nc.sync.dma_start(xt[:, :], xT_d[j, :, :])
                xts.append(xt)
                et = cpool.tile([128, TSLOTS * KCODES], F32, name=f"et{j}", tag=f"et{j}")
                nc.sync.dma_start(et[:, :], etsel_d[j, :, :])
                etl.append(et)
            esl = []
            for t in range(TSLOTS):
                es = cpool.tile([128, D + 1], F32, name=f"es{t}", tag=f"es{t}")
                nc.sync.dma_start(es[:, :], esel_d[t, :, :])
                esl.append(es)
            entl = []
            for j in range(2):
                en = cpool.tile([128, NE], BF16, name=f"en{j}", tag=f"en{j}")
                nc.sync.dma_start(en[:, :], entp_d[j, :, :])
                entl.append(en)

            # ---- quantize: per type-slot ----
            for t in range(TSLOTS):
                ksl = slice(t * KCODES, (t + 1) * KCODES)
                dps = psq.tile([128, S], F32, tag="dps")
                nc.tensor.matmul(dps[:, :], etl[0][:, ksl], xts[0][:, t * S:(t + 1) * S],
                                 start=True, stop=False)
                nc.tensor.matmul(dps[:, :], etl[1][:, ksl], xts[1][:, t * S:(t + 1) * S],
                                 start=False, stop=True)
                expT = wq.tile([128, S], F32, tag="expT")
                nc.scalar.activation(expT[:, :], dps[:, :], EXP)
                for c in range(NCH):
                    csl = slice(c * 128, (c + 1) * 128)
                    # argmax path: transpose exp tile so tokens sit on partitions
                    trp = psq.tile([128, 128], F32, tag="trp")
                    nc.tensor.transpose(trp[:, :], expT[:, csl], idf32[:, :])
                    trs = wq.tile([128, 128], F32, tag="trs")
                    nc.vector.tensor_copy(trs[:, :], trp[:, :])
                    mx8 = wq.tile([128, 8], F32, tag="mx8")
                    mi8 = wq.tile([128, 8], U32, tag="mi8")
                    nc.vector.max(mx8[:, :], trs[:, :])
                    nc.vector.max_index(mi8[:, :], mx8[:, :], trs[:, :])
                    nc.sync.dma_start(idx_d[t, c, 0:128], mi8[:, 0:1])
                    # z_q path: [tok, 257] = expT_chunk.T @ [E_t | 1]
                    zps = psq.tile([128, D + 1], F32, tag="zps")
                    nc.tensor.matmul(zps[:, :], expT[:, csl], esl[t][:, :],
                                     start=True, stop=True)
                    rec = wq.tile([128, 1], F32, tag="rec")
                    nc.vector.tensor_copy(rec[:, :], zps[:, D:D + 1])
                    nc.vector.reciprocal(rec[:, :], rec[:, :])
                    zsb = wq.tile([128, D], F32, tag="zsb")
                    nc.vector.tensor_scalar_mul(zsb[:, :], zps[:, 0:D], rec[:, :])
                    nc.sync.dma_start(zq_d[t, csl, :], zsb[:, :])

            # ---- uniform loss: per block-slot ----
            uacc = cpool.tile([128, 2 * TSLOTS], F32)
            invt = float(1.0 / TEMPERATURE)
            for s in range(TSLOTS):
                wsl = slice(s * 128, (s + 1) * 128)
                sums = uw.tile([128, 8], F32, tag="usums")
                for c in range(NCHUNKS):
                    w = min(UCHUNK, NE - c * UCHUNK)
                    ups = psu.tile([128, w], F32, tag="ups")
                    nc.tensor.matmul(ups[:, :], entl[0][:, wsl],
                                     entl[0][:, c * UCHUNK:c * UCHUNK + w],
                                     start=True, stop=False)
                    nc.tensor.matmul(ups[:, :], entl[1][:, wsl],
                                     entl[1][:, c * UCHUNK:c * UCHUNK + w],
                                     start=False, stop=True)
                    if c == 0:
                        # kill diagonal (own window = cols [s*128, s*128+128))
                        nc.vector.copy_predicated(ups[:, wsl], eye8[:, :], negt[:, :])
                        expw = uw.tile([128, 128], BF16, tag="uexpw")
                        nc.scalar.activation(expw[:, :], ups[:, wsl], EXP, scale=invt,
                                             accum_out=uacc[:, 2 * s:2 * s + 1])
                    scr = uw.tile([128, w], BF16, tag="uscr")
                    nc.scalar.activation(scr[:, :], ups[:, :], EXP, scale=invt,
                                         accum_out=sums[:, c:c + 1])
                nc.vector.reduce_sum(uacc[:, 2 * s + 1:2 * s + 2], sums[:, 0:NCHUNKS],
                                     axis=mybir.AxisListType.X)
            nc.sync.dma_start(uout_d[:, :], uacc[:, :])
    nc.compile()
    return nc


def _prep_inputs(x, Q, E, S):
    """Per-core host prep. Returns (in_maps, slot_tokens)."""
    En = E / np.linalg.norm(E, axis=1, keepdims=True)
    EnT = np.ascontiguousarray(En.T)  # [256, 3328]
    in_maps = []
    slot_tokens = []
    for c in range(NCORES):
        toks = []
        xg = np.zeros((TSLOTS * S, D), dtype=np.float32)
        esel = np.zeros((TSLOTS, 128, D + 1), dtype=np.float32)
        esel[:, :, D] = 1.0
        ets = np.zeros((TSLOTS * KCODES, D), dtype=np.float32)
        for t in range(TSLOTS):
            g = SLOT_MAP[c][t]
            if g < 0:
                toks.append(np.empty(0, dtype=np.int64))
                continue
            tk = np.nonzero(Q == g)[0]
            toks.append(tk)
            xg[t * S:t * S + len(tk)] = x[tk]
            esel[t, :, :D] = E[g * KCODES:(g + 1) * KCODES]
            ets[t * KCODES:(t + 1) * KCODES] = E[g * KCODES:(g + 1) * KCODES]
        slot_tokens.append(toks)
        xT = np.ascontiguousarray(xg.T.reshape(2, 128, TSLOTS * S))
        etsel = np.ascontiguousarray(ets.T.reshape(2, 128, TSLOTS * KCODES))
        # column-permuted EnT: own 4 blocks first, then the rest
        own = [g for g in SLOT_MAP[c] if g >= 0]
        rest = [b for b in range(NUM_TYPES) if b not in own]
        # dummy slots borrow a real block from rest (their pos/tot outputs are
        # discarded) so slot s always maps to perm cols [s*128, (s+1)*128)
        npad = TSLOTS - len(own)
        order = own + rest[:npad] + rest[npad:]
        cols = np.concatenate([np.arange(b * KCODES, (b + 1) * KCODES) for b in order])
        entp = np.ascontiguousarray(
            EnT[:, cols].reshape(2, 128, NE)).astype(ml_dtypes.bfloat16)
        in_maps.append({"xT": xT, "esel": esel, "etsel": etsel, "entp": entp})
    return in_maps, slot_tokens


def kernel(x, Q, E, tau):
    from concourse.bass_utils import run_bass_kernel_spmd

    x = np.asarray(x, dtype=np.float32)
    Q = np.asarray(Q)
    E = np.asarray(E, dtype=np.float32)
    tauf = float(np.asarray(tau))
    if tauf != 1.0:
        x = x / np.float32(tauf)

    counts = np.bincount(Q.astype(np.int64), minlength=NUM_TYPES)
    S = max(256, int(-(-counts.max() // 128)) * 128)

    if S not in _cache:
        _cache[S] = _build(S)
    nc = _cache[S]

    in_maps, slot_tokens = _prep_inputs(x, Q, E, S)
    res = run_bass_kernel_spmd(nc, in_maps, core_ids=list(range(NCORES)))

    idx_dtype = np.result_type(np.int32, Q.dtype)
    quantized = np.zeros((x.shape[0], D), dtype=np.float32)
    encoding = np.zeros(x.shape[0], dtype=idx_dtype)
    rvals = np.zeros(NE, dtype=np.float64)
    for c in range(NCORES):
        r = res.results[c]
        zq = r["zq"]          # [TSLOTS, S, D]
        idx = r["idx"].reshape(TSLOTS, S)
        uout = r["uout"]      # [128, 8]
        for t in range(TSLOTS):
            g = SLOT_MAP[c][t]
            if g < 0:
                continue
            tk = slot_tokens[c][t]
            m = len(tk)
            if m:
                quantized[tk] = zq[t, :m]
                encoding[tk] = (idx[t, :m].astype(np.int64) + g * KCODES).astype(idx_dtype)
            pos = uout[:, 2 * t].astype(np.float64)
            tot = uout[:, 2 * t + 1].astype(np.float64)
            rvals[g * KCODES:(g + 1) * KCODES] = np.log(pos / tot)
    uniform_loss = np.float32(-rvals.mean())
    return quantized, np.float32(0.0), uniform_loss, encoding
